# revision 10
# baseline (speedup 1.0000x reference)
"""D-MPNN layer on 8 TRN2 NeuronCores (Bass/Tile, SPMD) — v3.

out = (1-z)*s + z*m with
  mess_ki = mess[nei_idx]                       [M, D]
  s_ij    = segment_sum(mess_ki, src_idx, E)    [E, D]
  z_ij    = sigmoid([h_ij | s_ij] @ Wz + bz)    [E, D]
  r_ki    = sigmoid([h_ki | mess_ki] @ Wr + br) [M, D]
  r_ij    = segment_sum(r_ki*mess_ki, src, E)   [E, D]
  m_ij    = tanh(h_ij @ W + bw + r_ij @ U)      [E, D]

Sharding: edges E split into 8 contiguous chunks (EC=E/8); each M-row is
routed on host to the core owning its src edge, so segment sums are
core-local (no collectives).  Rows (sorted by src) are greedily packed into
variable-width dst blocks (window <= 128 dst edges, <= 384 rows, padded to
384); a final 4-tile block covers the core's last 128 dst edges.  One static
program for all cores (block count padded to a common B2).

v3 changes vs v2:
  - z/m preacts accumulate into one fused [z|m] PSUM region per block;
    weights streamed as fused [Wz|W] 512-wide rhs; hijb zero-padded to 128
    partitions so FWL stays on.  Sigmoid/tanh batched across both blocks of
    a group via a 2-bank PSUM tile.
  - h_ij shipped as fp8-e3m4 (safe precision), halving its HBM traffic.
  - s^T/r^T produced by DMA XBAR transpose (dma_start_transpose) straight
    from the bf16 copy of the agg PSUM — TensorE transpose matmuls and the
    second PSUM drain (c2) are gone.
  - aggregation split: mess aggregated per tile (bf16 rhs, fp8 onehot
    stationary - mixed dtype matmul), r*mess aggregated pairwise with a
    DoubleRow fp8 matmul (onehot pair stationary [128,2,128], rm pair rhs
    [128,2,256]) - contraction 256 rows per pass.
  - one fp8 onehot build (gpsimd) serves mess-agg, rm-single and rm-pair.
  - output in bf16; combine in bf16 split across DVE and gpsimd.
"""

import numpy as np
import ml_dtypes

BF16 = ml_dtypes.bfloat16
F8 = ml_dtypes.float8_e4m3
F8E3 = ml_dtypes.float8_e3m4

E = 262144
M = 786432
F_NB = 192
D = 256
NCORES = 8

FULL_DIMS = dict(E=E, M=M, F=F_NB, D=D, ncores=NCORES, BLK=128, C=384,
                 CT=512, KG=6)


def _dims(d, B2):
    o = dict(d)
    o["B2"] = B2
    o["EC"] = o["E"] // o["ncores"]
    o["TPB"] = o["C"] // 128              # 3 row tiles per normal block
    o["TPT"] = o["CT"] // 128             # 4 row tiles in the tail block
    assert o["KG"] == 2 * o["TPB"]
    assert B2 % 2 == 0
    o["G"] = B2 // 2
    o["T"] = o["TPB"] * B2 + o["TPT"]     # total row tiles per core
    return o


def _greedy_blocks(csum, EC, C):
    bases = []
    i = 0
    while i < EC - 128:
        base = i
        hi = min(base + 128, EC - 128)
        j = int(np.searchsorted(csum, csum[base] + C, side="right")) - 1
        j = max(base + 1, min(j, hi))
        bases.append(base)
        i = j
    return bases


def _f8(a):
    return np.clip(a, -240.0, 240.0).astype(F8)


def _interleave_w(w8, lo, hi, ki, ncol):
    """Weight rows [lo:hi) -> DoubleRow [ki, 2, ncol] -> [128, 2*ncol],
    pairing (lo+k, lo+ki+k)."""
    assert hi - lo == 2 * ki
    a = w8[lo:hi].reshape(2, ki, ncol).transpose(1, 0, 2).reshape(ki, 2 * ncol)
    out = np.zeros((128, 2 * ncol), F8)
    out[:ki] = a
    return out


def host_prep(inputs, dims=FULL_DIMS):
    dm0 = dict(dims)
    EC = dm0["E"] // dm0["ncores"]
    C, CT, KG = dm0["C"], dm0["CT"], dm0["KG"]
    F, Dd = dm0["F"], dm0["D"]
    ncores = dm0["ncores"]
    TPB = C // 128

    src = np.asarray(inputs["src_idx"]).astype(np.int64).ravel()
    nei = np.asarray(inputs["nei_idx"]).astype(np.int64).ravel()
    h_ij = np.asarray(inputs["h_ij"])
    h_ki = np.asarray(inputs["h_ki"])
    mess = np.asarray(inputs["mess"])

    order = np.argsort(src, kind="stable")
    src_s = src[order]
    cnt = np.bincount(src_s, minlength=dm0["E"])

    core_blocks = []
    for c in range(ncores):
        csum = np.concatenate(
            [[0], np.cumsum(cnt[c * EC:(c + 1) * EC])]
        )
        bases = _greedy_blocks(csum, EC, C)
        tail_rows = csum[EC] - csum[EC - 128]
        if tail_rows > CT:
            raise OverflowError(f"tail rows {tail_rows} > CT={CT}")
        core_blocks.append((bases, csum))
    nreal = [len(b[0]) for b in core_blocks]
    B2 = max(nreal)
    B2 += B2 % 2
    dm = _dims(dm0, B2)
    G, T = dm["G"], dm["T"]
    TPT = dm["TPT"]

    mess_bf = mess.astype(BF16)
    h_ki_s = h_ki[order]
    nei_s = nei[order]
    mess_g_all = mess_bf[nei_s]            # [M, D] gathered, src-sorted

    # ---- weights ----
    wr = np.asarray(inputs["Wr_w"]).astype(np.float32)   # [448, 256]
    wz = np.asarray(inputs["Wz_w"]).astype(np.float32)   # [448, 256]
    u = np.asarray(inputs["U_w"]).astype(np.float32)     # [256, 256]
    w = np.asarray(inputs["W_w"]).astype(np.float32)     # [192, 256]
    wr8 = _f8(wr)
    wz_b = wz.astype(BF16)
    w_b = w.astype(BF16)
    u_b = u.astype(BF16)
    # fused [Wz|W] streaming rhs for the h-part of z/m
    wzw_a = np.concatenate([wz_b[0:128], w_b[0:128]], axis=1)     # [128,512]
    wzw_b = np.zeros((128, 2 * Dd), BF16)
    wzw_b[0:64] = np.concatenate([wz_b[128:192], w_b[128:192]], axis=1)
    wmap = dict(
        wr_dr1=_interleave_w(wr8, 0, 256, 128, Dd),
        wr_dr2=_interleave_w(wr8, 256, 448, 96, Dd),
        wzw_a=np.ascontiguousarray(wzw_a),
        wzw_b=np.ascontiguousarray(wzw_b),
        wz2=np.ascontiguousarray(wz_b[192:320]),
        wz3=np.ascontiguousarray(wz_b[320:448]),
        u0=np.ascontiguousarray(u_b[0:128]),
        u1=np.ascontiguousarray(u_b[128:256]),
    )

    row_lo = np.searchsorted(src_s, np.arange(ncores) * EC)
    row_hi = np.searchsorted(src_s, (np.arange(ncores) + 1) * EC)

    in_maps = []
    metas = []
    for c in range(ncores):
        bases, csum = core_blocks[c]
        nb = len(bases)
        ndummy = B2 - nb
        MPC = B2 * C + CT
        rlo = row_lo[c]
        nrow_core = row_hi[c] - rlo

        bases_arr = np.asarray(bases, dtype=np.int64)
        nexts = np.concatenate([bases_arr[1:], [EC - 128]])
        widths = nexts - bases_arr
        rs = csum[bases_arr]               # first row of each block
        tail_start = csum[EC - 128]

        rowblk = np.zeros(nrow_core, np.int64)
        rowblk[rs[1:][rs[1:] < nrow_core]] += 1
        rowblk = np.cumsum(rowblk)
        blk_of_row = np.minimum(rowblk, nb - 1)
        ridx = np.arange(nrow_core)
        is_tail = ridx >= tail_start
        pos_in_blk = ridx - rs[blk_of_row]
        slot_of_row = np.where(
            is_tail,
            B2 * C + (ridx - tail_start),
            (ndummy + blk_of_row) * C + pos_in_blk,
        )
        base_of_row = np.where(is_tail, EC - 128, bases_arr[blk_of_row])
        srcrel_pad = np.full(MPC, 999.0, np.float32)
        srcrel_pad[slot_of_row] = (
            src_s[rlo:row_hi[c]] - c * EC - base_of_row
        ).astype(np.float32)

        # padded per-row data
        x_pad = np.zeros((MPC, F + Dd), np.float32)
        x_pad[slot_of_row, :F] = h_ki_s[rlo:row_hi[c]]
        x_pad[slot_of_row, F:] = mess_g_all[rlo:row_hi[c]].astype(np.float32)
        x8 = _f8(x_pad)                    # [MPC, 448] fp8
        mg_pad = np.zeros((MPC, Dd), BF16)
        mg_pad[slot_of_row] = mess_g_all[rlo:row_hi[c]]

        # h_ij^T per block as fp8-e3m4: [B2+1, 128, 256]
        #   cols 0:128   = h^T[0:128, dst]
        #   cols 128:256 = h^T[128:192, dst] in rows 0:64, rows 64:128 zero
        hijc = np.clip(h_ij[c * EC:(c + 1) * EC], -28.0, 28.0).astype(F8E3)
        gather_rows = bases_arr[:, None] + np.arange(128)[None, :]
        hij_all = np.zeros((B2 + 1, 128, F), F8E3)
        hij_all[ndummy:B2] = hijc[gather_rows]
        hij_all[B2] = hijc[EC - 128:]
        hijt = hij_all.transpose(0, 2, 1)  # [B2+1, 192, 128] e3m4
        bh8 = np.zeros((B2 + 1, 128, 256), F8E3)
        bh8[:, :, 0:128] = hijt[:, 0:128, :]
        bh8[:, 0:64, 128:256] = hijt[:, 128:192, :]

        # ---- per-tile fp8 X^T DoubleRow sections ----
        xt = x8[:T * 128].reshape(T, 128, F + Dd)
        xdr1 = (xt[:, :, 0:256].transpose(0, 2, 1)   # [T, 256f, 128r]
                .reshape(T, 2, 128, 128).transpose(0, 2, 1, 3)
                .reshape(T, 128, 256))
        x2 = (xt[:, :, 256:448].transpose(0, 2, 1)   # [T, 192f, 128r]
              .reshape(T, 2, 96, 128).transpose(0, 2, 1, 3)
              .reshape(T, 96, 256))
        xdr2 = np.zeros((T, 128, 256), F8)
        xdr2[:, :96] = x2

        # mess row-major bf16, tile-major: [T, 128, 256]
        mg_t = mg_pad.reshape(T, 128, Dd)

        # ---- group blobs ----
        NT = dm["TPB"] * B2                # tiles in normal blocks
        xdr1_g = xdr1[:NT].reshape(G, KG, 128, 256)
        xdr2_g = xdr2[:NT].reshape(G, KG, 128, 256)
        blob8 = np.concatenate([
            xdr1_g.transpose(0, 2, 1, 3).reshape(G, 128, KG * 256),
            xdr2_g.transpose(0, 2, 1, 3).reshape(G, 128, KG * 256),
        ], axis=2)
        # bf16 blob: mess only
        blobb = (mg_t[:NT].reshape(G, KG, 128, Dd)
                 .transpose(0, 2, 1, 3).reshape(G, 128, KG * Dd))
        bh8_g = (bh8[0:B2].reshape(G, 2, 128, 256)
                 .transpose(0, 2, 1, 3).reshape(G, 128, 512))

        # ---- tail sections (TPT=4 tiles, 1 block) ----
        t0 = NT
        tail8 = np.concatenate([
            xdr1[t0:].transpose(1, 0, 2).reshape(128, TPT * 256),
            xdr2[t0:].transpose(1, 0, 2).reshape(128, TPT * 256),
        ], axis=1)
        tailb = mg_t[t0:].transpose(1, 0, 2).reshape(128, TPT * Dd)
        tailh = bh8[B2]

        src_all = np.ascontiguousarray(srcrel_pad.reshape(T, 128).T)

        im = dict(srcrel=src_all,
                  blob8=np.ascontiguousarray(blob8),
                  blobb=np.ascontiguousarray(blobb),
                  bh8=np.ascontiguousarray(bh8_g),
                  tail8=np.ascontiguousarray(tail8),
                  tailb=np.ascontiguousarray(tailb),
                  tailh=np.ascontiguousarray(tailh))
        im.update(wmap)
        in_maps.append(im)
        metas.append(dict(bases=bases_arr, widths=widths, ndummy=ndummy))
    return in_maps, metas, dm


def build_program(dm):
    import concourse.tile as tile
    from concourse import bacc, mybir

    EC, KG, T, G, B2 = dm["EC"], dm["KG"], dm["T"], dm["G"], dm["B2"]
    TPB, TPT, F, Dd = dm["TPB"], dm["TPT"], dm["F"], dm["D"]
    f32 = mybir.dt.float32
    bf16 = mybir.dt.bfloat16
    fp8 = mybir.dt.float8e4
    fp8e3 = mybir.dt.float8e3
    i32 = mybir.dt.int32
    AF = mybir.ActivationFunctionType
    ALU = mybir.AluOpType
    DR = mybir.MatmulPerfMode.DoubleRow

    nc = bacc.Bacc("TRN2", target_bir_lowering=False, debug=False,
                   num_devices=dm["ncores"])

    NF8 = KG * 256 * 2
    NBF = KG * Dd
    NT8 = TPT * 256 * 2
    NTB = TPT * Dd

    srcrel_d = nc.dram_tensor("srcrel", [128, T], f32, kind="ExternalInput")
    blob8_d = nc.dram_tensor("blob8", [G, 128, NF8], fp8, kind="ExternalInput")
    blobb_d = nc.dram_tensor("blobb", [G, 128, NBF], bf16,
                             kind="ExternalInput")
    bh8_d = nc.dram_tensor("bh8", [G, 128, 512], fp8e3, kind="ExternalInput")
    tail8_d = nc.dram_tensor("tail8", [128, NT8], fp8, kind="ExternalInput")
    tailb_d = nc.dram_tensor("tailb", [128, NTB], bf16, kind="ExternalInput")
    tailh_d = nc.dram_tensor("tailh", [128, 256], fp8e3, kind="ExternalInput")
    wd8 = {n: nc.dram_tensor(n, [128, 512], fp8, kind="ExternalInput")
           for n in ("wr_dr1", "wr_dr2")}
    wdb = {n: nc.dram_tensor(n, [128, 512], bf16, kind="ExternalInput")
           for n in ("wzw_a", "wzw_b")}
    wdc = {n: nc.dram_tensor(n, [128, Dd], bf16, kind="ExternalInput")
           for n in ("wz2", "wz3", "u0", "u1")}
    y_d = nc.dram_tensor("y", [(B2 + 1) * 128, Dd], bf16,
                         kind="ExternalOutput")

    def dr3(ap, ko=2):
        return ap.rearrange("p (ko n) -> p ko n", ko=ko)

    with tile.TileContext(nc) as tc:
        with (
            tc.tile_pool(name="const", bufs=1) as const,
            tc.tile_pool(name="gat", bufs=3) as gat,
            tc.tile_pool(name="mid", bufs=3) as mid,
            tc.tile_pool(name="fin", bufs=3) as fin,
            tc.tile_pool(name="psPR", bufs=2, space="PSUM") as psPR,
            tc.tile_pool(name="psSR", bufs=2, space="PSUM") as psSR,
            tc.tile_pool(name="psZM", bufs=2, space="PSUM") as psZM,
        ):
            iota_i = const.tile([128, 128], i32)
            nc.gpsimd.iota(iota_i[:], pattern=[[1, 128]], base=0,
                           channel_multiplier=0)
            iota_f = const.tile([128, 128], f32)
            nc.vector.tensor_copy(iota_f[:], iota_i[:])

            wt = {}
            for n, dram in list(wd8.items()) + list(wdb.items()):
                t = const.tile([128, 512], fp8 if n in wd8 else bf16, tag=n)
                nc.sync.dma_start(out=t[:], in_=dram[:, :])
                wt[n] = t
            for n, dram in wdc.items():
                t = const.tile([128, Dd], bf16, tag=n)
                nc.sync.dma_start(out=t[:], in_=dram[:, :])
                wt[n] = t

            src_all = const.tile([128, T], f32)
            nc.sync.dma_start(out=src_all[:], in_=srcrel_d[:, :])

            def do_group(ntile, nblk, t0, b8, mg, bh, tag):
                """ntile row tiles, nblk dst blocks (<=2); 2 tiles pair per
                DR rm-agg; odd tile per block aggregated in bf16.
                b8: fp8 X^T sections; mg: [128, ntile, 256] bf16 mess;
                bh: [128, nblk, 256] e3m4 h^T."""
                x1o = 0
                x2o = ntile * 256
                TB = ntile // nblk         # tiles per block (3 or 4)
                npair = TB // 2            # DR pairs per block (1 or 2)

                # ---- onehot: one fp8 build serves everything (DVE;
                # Pool rejects is_equal) ----
                oh = mid.tile([128, KG, 128], fp8, tag="oh")
                nc.vector.tensor_tensor(
                    out=oh[:, :ntile, :],
                    in0=src_all[:, t0:t0 + ntile, None].broadcast_to(
                        [128, ntile, 128]),
                    in1=iota_f[:, None, :].broadcast_to([128, ntile, 128]),
                    op=ALU.is_equal,
                )

                # ---- r phase: fp8 DoubleRow ----
                r_g = mid.tile([128, KG * Dd], bf16, tag="rg")
                for jj in range(0, ntile, 2):
                    np2 = min(2, ntile - jj)
                    pr2 = psPR.tile([128, 512], f32, tag="pr2")
                    for q in range(np2):
                        j = jj + q
                        x1 = dr3(b8[:, x1o + j * 256:x1o + (j + 1) * 256])
                        x2 = dr3(b8[0:96, x2o + j * 256:x2o + (j + 1) * 256])
                        po = pr2[:, q * 256:(q + 1) * 256]
                        nc.tensor.matmul(out=po, lhsT=x1,
                                         rhs=dr3(wt["wr_dr1"][:]),
                                         start=True, stop=False, perf_mode=DR)
                        nc.tensor.matmul(out=po, lhsT=x2,
                                         rhs=dr3(wt["wr_dr2"][0:96, :]),
                                         start=False, stop=True, perf_mode=DR)
                    nc.scalar.activation(
                        r_g[:, jj * Dd:(jj + np2) * Dd],
                        pr2[:, :np2 * 256], AF.Sigmoid)

                # ---- rm products ----
                # pairs (first 2 tiles of each block) -> fp8 for DR agg
                rgv = r_g[:, :ntile * Dd].rearrange(
                    "p (b t d) -> p b t d", b=nblk, t=TB)
                mgv = mg[:, :ntile, :].rearrange(
                    "p (b t) d -> p b t d", b=nblk)
                rmp = mid.tile([128, 2, 2, 256], fp8, tag="rmp")
                nc.vector.tensor_tensor(
                    out=rmp[:, :nblk, :, :],
                    in0=rgv[:, :, 0:2, :],
                    in1=mgv[:, :, 0:2, :],
                    op=ALU.mult,
                )
                if npair == 2:
                    # tail: two pairs per block; rmp holds pair 0, rmp2 pair 1
                    rmp2 = mid.tile([128, 2, 2, 256], fp8, tag="rmp2")
                    nc.vector.tensor_tensor(
                        out=rmp2[:, :nblk, :, :],
                        in0=rgv[:, :, 2:4, :],
                        in1=mgv[:, :, 2:4, :],
                        op=ALU.mult,
                    )
                # odd single tile per block (TB==3) -> bf16 (on gpsimd)
                if TB % 2 == 1:
                    rms = mid.tile([128, 2, 256], bf16, tag="rms")
                    nc.gpsimd.tensor_tensor(
                        out=rms[:, :nblk, :],
                        in0=rgv[:, :, TB - 1, :],
                        in1=mgv[:, :, TB - 1, :],
                        op=ALU.mult,
                    )

                # ---- aggregation + transpose + z/m per block ----
                pzm = psZM.tile([128, 2, 512], f32, tag="pzm")
                c1g = fin.tile([128, 2, 512], bf16, tag="c1g")
                sT = []
                for bbk in range(nblk):
                    ps_sr = psSR.tile([128, 512], f32, tag="ps_sr")
                    base_t = bbk * TB
                    # mess aggregation, one bf16 matmul per tile
                    for tj in range(TB):
                        j = base_t + tj
                        nc.tensor.matmul(
                            out=ps_sr[:, 0:256], lhsT=oh[:, j, :],
                            rhs=mg[:, j, :],
                            start=(tj == 0), stop=False,
                            skip_group_check=True)
                    # rm aggregation: DR pairs + optional bf16 single
                    for pp in range(npair):
                        rmsrc = rmp if pp == 0 else rmp2
                        nc.tensor.matmul(
                            out=ps_sr[:, 256:512],
                            lhsT=oh[:, base_t + 2 * pp:base_t + 2 * pp + 2, :],
                            rhs=rmsrc[:, bbk, :, :],
                            start=(pp == 0), stop=(TB % 2 == 0
                                                   and pp == npair - 1),
                            perf_mode=DR, skip_group_check=True)
                    if TB % 2 == 1:
                        nc.tensor.matmul(
                            out=ps_sr[:, 256:512],
                            lhsT=oh[:, base_t + TB - 1, :],
                            rhs=rms[:, bbk, :],
                            start=False, stop=True,
                            skip_group_check=True)
                    # drain [s|r] to SBUF bf16 (split across Scalar/Vector)
                    if bbk == 0:
                        nc.scalar.activation(c1g[:, 0, :], ps_sr[:], AF.Copy)
                    else:
                        nc.vector.tensor_copy(c1g[:, bbk, :], ps_sr[:])
                    # XBAR transpose -> [s^T c0|c1, r^T c0|c1]
                    st = fin.tile([128, 4, 128], bf16, tag=f"sT{bbk}")
                    nc.sync.dma_start_transpose(out=st[:], in_=c1g[:, bbk, :])
                    sT.append(st)

                    # z/m preacts: fused [z|m] into one PSUM bank
                    po = pzm[:, bbk, :]
                    nc.tensor.matmul(out=po, lhsT=bh[:, bbk, 0:128],
                                     rhs=wt["wzw_a"][:],
                                     start=True, stop=False,
                                     skip_group_check=True)
                    nc.tensor.matmul(out=po, lhsT=bh[:, bbk, 128:256],
                                     rhs=wt["wzw_b"][:],
                                     start=False, stop=False,
                                     skip_group_check=True)
                    nc.tensor.matmul(out=po[:, 0:256], lhsT=st[:, 0, :],
                                     rhs=wt["wz2"][:], start=False, stop=False,
                                     skip_group_check=True)
                    nc.tensor.matmul(out=po[:, 0:256], lhsT=st[:, 1, :],
                                     rhs=wt["wz3"][:], start=False, stop=False,
                                     skip_group_check=True)
                    nc.tensor.matmul(out=po[:, 256:512], lhsT=st[:, 2, :],
                                     rhs=wt["u0"][:], start=False, stop=False,
                                     skip_group_check=True)
                    nc.tensor.matmul(out=po[:, 256:512], lhsT=st[:, 3, :],
                                     rhs=wt["u1"][:], start=False, stop=True,
                                     skip_group_check=True)

                # ---- activations batched across blocks ----
                z_sb = fin.tile([128, 2, 256], bf16, tag="z")
                nc.scalar.activation(z_sb[:, :nblk, :], pzm[:, :nblk, 0:256],
                                     AF.Sigmoid)
                m_sb = fin.tile([128, 2, 256], bf16, tag="m")
                nc.scalar.activation(m_sb[:, :nblk, :], pzm[:, :nblk, 256:512],
                                     AF.Tanh)

                # ---- combine (1-z)s + zm = s + z*(m-s), bf16 ----
                s_view = c1g[:, :nblk, 0:256]
                t1 = fin.tile([128, 2, 256], bf16, tag="t1")
                nc.vector.tensor_tensor(out=t1[:, :nblk, :],
                                        in0=m_sb[:, :nblk, :], in1=s_view,
                                        op=ALU.subtract)
                nc.gpsimd.tensor_tensor(out=t1[:, :nblk, :],
                                        in0=t1[:, :nblk, :],
                                        in1=z_sb[:, :nblk, :],
                                        op=ALU.mult)
                o_sb = fin.tile([128, 2, 256], bf16, tag="o")
                nc.gpsimd.tensor_tensor(out=o_sb[:, :nblk, :],
                                        in0=t1[:, :nblk, :], in1=s_view,
                                        op=ALU.add)
                return o_sb

            for g in range(G):
                b8 = gat.tile([128, NF8], fp8, tag="b8")
                nc.sync.dma_start(out=b8[:], in_=blob8_d[g])
                mg = mid.tile([128, KG, 256], bf16, tag="mg")
                nc.sync.dma_start(
                    out=mg[:],
                    in_=blobb_d[g][:, :].rearrange("p (j d) -> p j d", j=KG))
                bh = gat.tile([128, 2, 256], fp8e3, tag="bh")
                nc.sync.dma_start(
                    out=bh[:],
                    in_=bh8_d[g][:, :].rearrange("p (b d) -> p b d", b=2))
                o_sb = do_group(KG, 2, g * KG, b8, mg, bh, g)
                yv = y_d[2 * g * 128:(2 * g + 2) * 128, :].rearrange(
                    "(bb p) d -> p bb d", bb=2)
                nc.sync.dma_start(out=yv, in_=o_sb[:])

            # tail block (TPT tiles, 1 block, 2 DR pairs)
            t8 = gat.tile([128, NT8], fp8, tag="t8")
            nc.sync.dma_start(out=t8[:], in_=tail8_d[:, :])
            mg = mid.tile([128, KG, 256], bf16, tag="mg")
            nc.sync.dma_start(
                out=mg[:, 0:TPT, :],
                in_=tailb_d[:, :].rearrange("p (j d) -> p j d", j=TPT))
            bh = gat.tile([128, 2, 256], fp8e3, tag="bh")
            nc.sync.dma_start(out=bh[:, 0, :], in_=tailh_d[:, :])
            o_sb = do_group(TPT, 1, B2 * TPB, t8, mg, bh, G)
            nc.sync.dma_start(out=y_d[B2 * 128:(B2 + 1) * 128, :],
                              in_=o_sb[:, 0, :])

    nc.compile()
    return nc


_CACHE = {}
LAST_RESULT = None


def kernel(**inputs):
    from concourse.bass_utils import run_bass_kernel_spmd

    for b in ("Wz_b", "Wr_b", "W_b"):
        assert not np.any(np.asarray(inputs[b])), f"nonzero bias {b} unsupported"

    in_maps, metas, dm = host_prep(inputs, FULL_DIMS)
    key = (tuple(sorted(FULL_DIMS.items())), dm["B2"])
    if key not in _CACHE:
        _CACHE[key] = build_program(dm)
    nc = _CACHE[key]
    import os
    trace = os.environ.get("DMPNN_TRACE", "") == "1"
    res = run_bass_kernel_spmd(nc, in_maps, core_ids=list(range(dm["ncores"])),
                               trace=trace, trace_cores=[0] if trace else None)
    global LAST_RESULT
    LAST_RESULT = res

    EC = dm["EC"]
    B2 = dm["B2"]
    out = np.empty((dm["E"], dm["D"]), np.float32)
    for c in range(dm["ncores"]):
        yc = res.results[c]["y"].astype(np.float32)
        meta = metas[c]
        oc = out[c * EC:(c + 1) * EC]
        nd = meta["ndummy"]
        for i, (base, wdt) in enumerate(zip(meta["bases"], meta["widths"])):
            b = nd + i
            oc[base:base + wdt] = yc[b * 128:b * 128 + wdt]
        oc[EC - 128:] = yc[B2 * 128:(B2 + 1) * 128]
    return out
